# revision 15
# baseline (speedup 1.0000x reference)
"""Trainium2 Bass kernel: 7x7 valid cross-correlation (Conv2D) + bias on a
4096x4096 fp32 image, column-strip sharded over 8 NeuronCores (512 output
cols each, with a 6-col halo in each core's input strip).

Algorithm per core:
  - Output rows are processed in 34 tiles of 122 (=128-6) rows; each tile is
    one 512-wide PSUM chunk covering the core's whole column strip.
  - The 2D conv is 7 accumulating TensorE matmuls (one per horizontal tap b):
        psum[m, n] += B_b.T @ x[:, n+b]
    where B_b[k, m] = w[k-m, b] is a banded [128 x 122] matrix performing the
    7-tap vertical convolution for kernel column b.
  - Data is bf16 (inputs converted host-side, output upconverted host-side):
    halves DMA bytes and PE streams bf16 at 1 col/cycle with no f32r cast.
  - Every DMA touches a fully contiguous DRAM range (full-width rows), which
    lets the HW DGE fan packets across all 16 DMA engines; strided DRAM
    writes collapse onto 2 engines (~50 GB/s) and were the old bottleneck.
  - PSUM is evacuated by VectorE with a fused bias add + bf16 downcast.
    x loads go on the sync-engine HWDGE queue, y stores on the scalar-engine
    queue so in/out traffic never serializes at the queue level.
"""

import sys

sys.path.insert(0, "/opt/trn_rl_repo")

import ml_dtypes
import numpy as np

import concourse.bass as bass
import concourse.bacc as bacc
import concourse.mybir as mybir
from concourse.tile import TileContext
from concourse.bass_utils import run_bass_kernel_spmd

KH, KW = 7, 7
H, W = 4096, 4096
OH, OW = H - KH + 1, W - KW + 1  # 4090, 4090

NCORES = 8
CORE_OC = 512                    # output cols per core (core 7: 506 valid)
CORE_IC = CORE_OC + KH - 1       # 518 input cols needed
CORE_IC_PAD = 528                # pad rows to 1056B (32B-aligned, contiguous)
TILE_R = 128 - (KH - 1)          # 122 output rows per row-tile
N_TILES = -(-OH // TILE_R)       # 34
# Output tiles are staged in SBUF in groups and stored partition-major
# (DRAM row = SBUF partition), so each SWDGE store emits G*1KB-contiguous
# descriptors per partition instead of 1KB rows: 1KB descriptors have
# ~70-100ns/packet overhead that saturates the 16 SDMA engines and makes
# per-store completion lag by 5-10us. Trailing groups of 1 keep the
# end-of-kernel drain small.
GROUPS = [8, 8, 8, 8, 1, 1]      # tiles per store group (sum = 34)
GMAX = max(GROUPS)
N_GROUPS = len(GROUPS)
WARMUP_MMS = 5                   # ~3.2us of cold-rate N=512 matmuls flips HAM to 2.4GHz

BF16 = ml_dtypes.bfloat16

_NC_CACHE = {}


def _build_nc():
    f32 = mybir.dt.float32
    bf16 = mybir.dt.bfloat16
    kin = TILE_R + KH - 1  # 128
    assert kin == 128

    nc = bacc.Bacc()
    x_in = nc.declare_dram_parameter("x_in", [H, CORE_IC_PAD], bf16, isOutput=False)
    bands = nc.declare_dram_parameter("bands", [kin, KW * TILE_R], bf16, isOutput=False)
    biasb = nc.declare_dram_parameter("biasb", [128, 1], f32, isOutput=False)
    # partition-major: y_out[g, p, j*OC:(j+1)*OC] = output row TILE_R*t + p of
    # tile t = group_start(g) + j  (host unscrambles)
    y_out = nc.declare_dram_parameter(
        "y_out", [N_GROUPS, 128, GMAX * CORE_OC], bf16, isOutput=True
    )

    with TileContext(nc) as tc:
        with (
            tc.tile_pool(name="const", bufs=1) as cpool,
            tc.tile_pool(name="warm", bufs=1, space="PSUM") as wpool,
            tc.tile_pool(name="xio", bufs=8) as xpool,
            tc.tile_pool(name="yio", bufs=4) as ypool,
            tc.tile_pool(name="ps", bufs=7, space="PSUM") as ppool,
        ):
            band_sb = cpool.tile([kin, KW * TILE_R], bf16)
            bias_sb = cpool.tile([128, 1], f32)
            warm_sb = cpool.tile([128, CORE_OC], bf16)
            warm_ps = wpool.tile([128, CORE_OC], f32)
            # Warm-up: PE sits idle ~4us during the fixed preamble + first
            # loads; dependency-free matmuls on a memset tile flip the HAM
            # clock gate to 2.4GHz before the first real matmul issues.
            # N=512 keeps the duty cycle high enough to flip the activity
            # window (N=128 warmups measured too sparse to ever flip it).
            nc.vector.memset(warm_sb[:, :], 0)
            for _ in range(WARMUP_MMS):
                nc.tensor.matmul(
                    warm_ps[:, :],
                    lhsT=warm_sb[:, :128],
                    rhs=warm_sb[:, :],
                    start=True,
                    stop=True,
                )

            t = 0
            for g, gsz in enumerate(GROUPS):
                y_sb = ypool.tile([128, GMAX * CORE_OC], bf16, tag="y")
                for j in range(gsz):
                    r0 = t * TILE_R
                    h = min(TILE_R, OH - r0)
                    kh = h + KH - 1
                    x_sb = xpool.tile([kin, CORE_IC_PAD], bf16, tag="x")
                    if t == 0:
                        # tile 0 gates the first real matmul: split it across
                        # both HWDGE queues to halve its in-flight latency, and
                        # keep band/bias off the critical queues (SWDGE).
                        nc.sync.dma_start(out=x_sb[:64, :], in_=x_in[:64, :])
                        nc.scalar.dma_start(out=x_sb[64:kh, :], in_=x_in[64:kh, :])
                        nc.gpsimd.dma_start(out=band_sb[:, :], in_=bands[:, :])
                        nc.gpsimd.dma_start(out=bias_sb[:, :], in_=biasb[:, :])
                    else:
                        ldq = nc.sync if t % 2 == 0 else nc.scalar
                        ldq.dma_start(out=x_sb[:kh, :], in_=x_in[r0 : r0 + kh, :])
                    ps = ppool.tile([128, CORE_OC], f32, tag="ps")
                    for b in range(KW):
                        nc.tensor.matmul(
                            ps[:h, :],
                            lhsT=band_sb[:kh, b * TILE_R : b * TILE_R + h],
                            rhs=x_sb[:kh, b : b + CORE_OC],
                            start=(b == 0),
                            stop=(b == KW - 1),
                        )
                    nc.vector.tensor_scalar_add(
                        y_sb[:h, j * CORE_OC : (j + 1) * CORE_OC],
                        ps[:h, :],
                        bias_sb[:h, 0:1],
                    )
                    t += 1
                # Tiny trailing stores go HWDGE (scalar): 64KB fits its
                # 2-engine limit and completes faster than a SWDGE round trip,
                # shortening the end-of-kernel drain.
                stq = nc.gpsimd if gsz > 1 else nc.scalar
                stq.dma_start(
                    out=y_out[g, :, : gsz * CORE_OC], in_=y_sb[:, : gsz * CORE_OC]
                )
    nc.compile()
    return nc


def _make_bands(weight):
    """B_b[k, m] = w[k-m, b] laid out as [kin, KW*TILE_R] (band b in cols
    [b*TILE_R, (b+1)*TILE_R))."""
    kin = TILE_R + KH - 1
    bands = np.zeros((kin, KW * TILE_R), np.float32)
    m = np.arange(TILE_R)
    for b in range(KW):
        for a in range(KH):
            bands[m + a, b * TILE_R + m] = weight[a, b]
    return bands.astype(BF16)


def _shard_inputs(x, weight, bias):
    bands = _make_bands(weight)
    biasb = np.full((128, 1), np.float32(bias[0]), np.float32)
    xb = x.astype(BF16)
    in_maps = []
    for c in range(NCORES):
        c0 = c * CORE_OC
        cc = min(CORE_IC, W - c0)
        xt = np.zeros((H, CORE_IC_PAD), BF16)
        xt[:, :cc] = xb[:, c0 : c0 + cc]
        in_maps.append({"x_in": xt, "bands": bands, "biasb": biasb})
    return in_maps


def _assemble(results):
    out = np.empty((OH, OW), np.float32)
    for c in range(NCORES):
        c0 = c * CORE_OC
        cw = min(CORE_OC, OW - c0)
        y = results[c]["y_out"]  # [N_GROUPS, 128, GMAX*CORE_OC]
        strip = np.empty((OH, CORE_OC), np.float32)
        t = 0
        for g, gsz in enumerate(GROUPS):
            for j in range(gsz):
                r0 = t * TILE_R
                h = min(TILE_R, OH - r0)
                strip[r0 : r0 + h, :] = y[g, :h, j * CORE_OC : (j + 1) * CORE_OC]
                t += 1
        out[:, c0 : c0 + cw] = strip[:, :cw]
    return out


def _get_nc():
    key = (CORE_OC, TILE_R)
    if key not in _NC_CACHE:
        _NC_CACHE[key] = _build_nc()
    return _NC_CACHE[key]


def _run(x, weight, bias, **spmd_kwargs):
    x = np.ascontiguousarray(np.asarray(x), dtype=np.float32)
    weight = np.asarray(weight, dtype=np.float32)
    bias = np.asarray(bias, dtype=np.float32)
    in_maps = _shard_inputs(x, weight, bias)
    res = run_bass_kernel_spmd(_get_nc(), in_maps, list(range(NCORES)), **spmd_kwargs)
    return _assemble(res.results), res


def kernel(x, weight, bias):
    out, _ = _run(x, weight, bias)
    return out


# revision 17
# speedup vs baseline: 1.0313x; 1.0313x over previous
"""Trainium2 Bass kernel: 7x7 valid cross-correlation (Conv2D) + bias on a
4096x4096 fp32 image, column-strip sharded over 8 NeuronCores (512 output
cols each, with a 6-col halo in each core's input strip).

Algorithm per core:
  - Output rows are processed in 34 tiles of 122 (=128-6) rows; each tile is
    one 512-wide PSUM chunk covering the core's whole column strip.
  - The 2D conv is 7 accumulating TensorE matmuls (one per horizontal tap b):
        psum[m, n] += B_b.T @ x[:, n+b]
    where B_b[k, m] = w[k-m, b] is a banded [128 x 122] matrix performing the
    7-tap vertical convolution for kernel column b.
  - Data is bf16 (inputs converted host-side, output upconverted host-side):
    halves DMA bytes and PE streams bf16 at 1 col/cycle with no f32r cast.
  - Every DMA touches a fully contiguous DRAM range (full-width rows), which
    lets the HW DGE fan packets across all 16 DMA engines; strided DRAM
    writes collapse onto 2 engines (~50 GB/s) and were the old bottleneck.
  - PSUM is evacuated by VectorE with a fused bias add + bf16 downcast.
    x loads go on the sync-engine HWDGE queue, y stores on the scalar-engine
    queue so in/out traffic never serializes at the queue level.
"""

import sys

sys.path.insert(0, "/opt/trn_rl_repo")

import ml_dtypes
import numpy as np

import concourse.bass as bass
import concourse.bacc as bacc
import concourse.mybir as mybir
from concourse.tile import TileContext
from concourse.bass_utils import run_bass_kernel_spmd

KH, KW = 7, 7
H, W = 4096, 4096
OH, OW = H - KH + 1, W - KW + 1  # 4090, 4090

NCORES = 8
CORE_OC = 512                    # output cols per core (core 7: 506 valid)
CORE_IC = CORE_OC + KH - 1       # 518 input cols needed
CORE_IC_PAD = 528                # pad rows to 1056B (32B-aligned, contiguous)
TILE_R = 128 - (KH - 1)          # 122 output rows per row-tile
N_TILES = -(-OH // TILE_R)       # 34
# Output tiles are staged in SBUF in groups and stored partition-major
# (DRAM row = SBUF partition), so each SWDGE store emits G*1KB-contiguous
# descriptors per partition instead of 1KB rows: 1KB descriptors have
# ~70-100ns/packet overhead that saturates the 16 SDMA engines and makes
# per-store completion lag by 5-10us. Trailing groups of 1 keep the
# end-of-kernel drain small.
GROUPS = [8, 8, 8, 8, 1, 1]      # tiles per store group (sum = 34)
GMAX = max(GROUPS)
N_GROUPS = len(GROUPS)
WARMUP_MMS = 6                   # ~3.8us of cold-rate N=512 matmuls flips HAM to 2.4GHz

BF16 = ml_dtypes.bfloat16

_NC_CACHE = {}


def _build_nc():
    f32 = mybir.dt.float32
    bf16 = mybir.dt.bfloat16
    kin = TILE_R + KH - 1  # 128
    assert kin == 128

    nc = bacc.Bacc()
    x_in = nc.declare_dram_parameter("x_in", [H, CORE_IC_PAD], bf16, isOutput=False)
    bands = nc.declare_dram_parameter("bands", [kin, KW * TILE_R], bf16, isOutput=False)
    biasb = nc.declare_dram_parameter("biasb", [128, 1], f32, isOutput=False)
    # partition-major: y_out[g, p, j*OC:(j+1)*OC] = output row TILE_R*t + p of
    # tile t = group_start(g) + j  (host unscrambles)
    y_out = nc.declare_dram_parameter(
        "y_out", [N_GROUPS, 128, GMAX * CORE_OC], bf16, isOutput=True
    )

    with TileContext(nc) as tc:
        with (
            tc.tile_pool(name="const", bufs=1) as cpool,
            tc.tile_pool(name="warm", bufs=1, space="PSUM") as wpool,
            tc.tile_pool(name="xio", bufs=8) as xpool,
            tc.tile_pool(name="yio", bufs=4) as ypool,
            tc.tile_pool(name="ps", bufs=7, space="PSUM") as ppool,
        ):
            band_sb = cpool.tile([kin, KW * TILE_R], bf16)
            bias_sb = cpool.tile([128, 1], f32)
            warm_sb = cpool.tile([128, CORE_OC], bf16)
            warm_ps = wpool.tile([128, CORE_OC], f32)
            # Warm-up: PE sits idle ~4us during the fixed preamble + first
            # loads; dependency-free matmuls on a memset tile flip the HAM
            # clock gate to 2.4GHz before the first real matmul issues.
            # N=512 keeps the duty cycle high enough to flip the activity
            # window (N=128 warmups measured too sparse to ever flip it).
            nc.gpsimd.memset(warm_sb[:, :], 0)
            for _ in range(WARMUP_MMS):
                nc.tensor.matmul(
                    warm_ps[:, :],
                    lhsT=warm_sb[:, :128],
                    rhs=warm_sb[:, :],
                    start=True,
                    stop=True,
                )

            t = 0
            for g, gsz in enumerate(GROUPS):
                y_sb = ypool.tile([128, GMAX * CORE_OC], bf16, tag="y")
                for j in range(gsz):
                    r0 = t * TILE_R
                    h = min(TILE_R, OH - r0)
                    kh = h + KH - 1
                    x_sb = xpool.tile([kin, CORE_IC_PAD], bf16, tag="x")
                    if t == 0:
                        # tile 0 gates the first real matmul: split it across
                        # both HWDGE queues to halve its in-flight latency, and
                        # keep band/bias off the critical queues (SWDGE).
                        nc.sync.dma_start(out=x_sb[:64, :], in_=x_in[:64, :])
                        nc.scalar.dma_start(out=x_sb[64:kh, :], in_=x_in[64:kh, :])
                        nc.gpsimd.dma_start(out=band_sb[:, :], in_=bands[:, :])
                        nc.gpsimd.dma_start(out=bias_sb[:, :], in_=biasb[:, :])
                    else:
                        ldq = nc.sync if t % 2 == 0 else nc.scalar
                        ldq.dma_start(out=x_sb[:kh, :], in_=x_in[r0 : r0 + kh, :])
                    ps = ppool.tile([128, CORE_OC], f32, tag="ps")
                    for b in range(KW):
                        nc.tensor.matmul(
                            ps[:h, :],
                            lhsT=band_sb[:kh, b * TILE_R : b * TILE_R + h],
                            rhs=x_sb[:kh, b : b + CORE_OC],
                            start=(b == 0),
                            stop=(b == KW - 1),
                        )
                    nc.vector.tensor_scalar_add(
                        y_sb[:h, j * CORE_OC : (j + 1) * CORE_OC],
                        ps[:h, :],
                        bias_sb[:h, 0:1],
                    )
                    t += 1
                # Tiny trailing stores go HWDGE (scalar): 64KB fits its
                # 2-engine limit and completes faster than a SWDGE round trip,
                # shortening the end-of-kernel drain.
                stq = nc.gpsimd if gsz > 1 else nc.scalar
                stq.dma_start(
                    out=y_out[g, :, : gsz * CORE_OC], in_=y_sb[:, : gsz * CORE_OC]
                )
    nc.compile()
    return nc


def _make_bands(weight):
    """B_b[k, m] = w[k-m, b] laid out as [kin, KW*TILE_R] (band b in cols
    [b*TILE_R, (b+1)*TILE_R))."""
    kin = TILE_R + KH - 1
    bands = np.zeros((kin, KW * TILE_R), np.float32)
    m = np.arange(TILE_R)
    for b in range(KW):
        for a in range(KH):
            bands[m + a, b * TILE_R + m] = weight[a, b]
    return bands.astype(BF16)


def _shard_inputs(x, weight, bias):
    bands = _make_bands(weight)
    biasb = np.full((128, 1), np.float32(bias[0]), np.float32)
    xb = x.astype(BF16)
    in_maps = []
    for c in range(NCORES):
        c0 = c * CORE_OC
        cc = min(CORE_IC, W - c0)
        xt = np.zeros((H, CORE_IC_PAD), BF16)
        xt[:, :cc] = xb[:, c0 : c0 + cc]
        in_maps.append({"x_in": xt, "bands": bands, "biasb": biasb})
    return in_maps


def _assemble(results):
    out = np.empty((OH, OW), np.float32)
    for c in range(NCORES):
        c0 = c * CORE_OC
        cw = min(CORE_OC, OW - c0)
        y = results[c]["y_out"]  # [N_GROUPS, 128, GMAX*CORE_OC]
        strip = np.empty((OH, CORE_OC), np.float32)
        t = 0
        for g, gsz in enumerate(GROUPS):
            for j in range(gsz):
                r0 = t * TILE_R
                h = min(TILE_R, OH - r0)
                strip[r0 : r0 + h, :] = y[g, :h, j * CORE_OC : (j + 1) * CORE_OC]
                t += 1
        out[:, c0 : c0 + cw] = strip[:, :cw]
    return out


def _get_nc():
    key = (CORE_OC, TILE_R)
    if key not in _NC_CACHE:
        _NC_CACHE[key] = _build_nc()
    return _NC_CACHE[key]


def _run(x, weight, bias, **spmd_kwargs):
    x = np.ascontiguousarray(np.asarray(x), dtype=np.float32)
    weight = np.asarray(weight, dtype=np.float32)
    bias = np.asarray(bias, dtype=np.float32)
    in_maps = _shard_inputs(x, weight, bias)
    res = run_bass_kernel_spmd(_get_nc(), in_maps, list(range(NCORES)), **spmd_kwargs)
    return _assemble(res.results), res


def kernel(x, weight, bias):
    out, _ = _run(x, weight, bias)
    return out
